# revision 18
# baseline (speedup 1.0000x reference)
"""Trainium2 Bass kernel for nn_CrossModal_Ranked_Attention.

Math (per batch row b, reference in fp32):
  p_T  = x_T  @ Wt  + bt          [300]
  p_IM = x_IM @ Wim + bim         [300]
  p_CD = x_CD @ Wt  + bt          [300]
  For branch X with (Wq, bq, Wk, bk):
    q = p Wq + bq ; k = p Wk + bk ; alpha = (q.k)/sqrt(300); Z = sigmoid(alpha)
  Using q.k = p.(A p + v) + c with A = Wq Wk^T, v = Wk bq + Wq bk, c = bq.bk
  m1 = ZI*ZT, m2 = ZCD*ZT ; softmax over {m1,m2} = sigmoid(+-(m1-m2))
  out = (p_T, a1 * p_IM, a2 * p_CD)

Mapping: pure data parallel over 8 cores (8192 rows each). On-chip layout is
feature-major ("transposed world"): activations live as [feat, batch] so the
TensorE contraction dim (partitions) is the feature dim. Host pre-transposes
the input shards and re-transposes the outputs. Matmuls run in fp16
(full rate, 11-bit mantissa, ~1e-3 rel err; fp32r fallback via KDT=f32r).
Batch is processed in 16 column-tiles of 512; each tile's scalar epilogue is
emitted one tile late so the PE always has dense projection work queued.
The two 44-row third chunks of the T/CD projections are col-tiled into one
PSUM tile and issued back-to-back so they run concurrently on disjoint PE
column groups.
"""
import os
from contextlib import ExitStack

import numpy as np

import concourse.bacc as bacc
import concourse.tile as tile
from concourse import mybir
from concourse.bass_utils import run_bass_kernel_spmd

B, D_T, D_IM, D = 65536, 768, 2048, 300
N_CORES = 8
BSH = B // N_CORES          # 8192 rows per core
NB = 512                    # batch columns per tile
NT = BSH // NB              # 16 tiles
MCH = [(0, 128), (128, 256), (256, 300)]
KT = D_T // 128             # 6
KI = D_IM // 128            # 16
INV_SQRT_D = float(np.float32(1.0) / np.sqrt(np.float32(D)))

F32R = mybir.dt.float32r
F32 = mybir.dt.float32

if os.environ.get("KDT", "f16") == "f32r":
    DT = F32R
    NPDT = np.float32
    PAIR = False
else:
    DT = mybir.dt.float16
    NPDT = np.float16
    PAIR = os.environ.get("KPAIR", "1") == "1"

_compiled = {}


def _build():
    nc = bacc.Bacc("TRN2", target_bir_lowering=False, debug=False,
                   num_devices=N_CORES)
    xt_t = nc.dram_tensor("xt_t", [D_T, BSH], DT, kind="ExternalInput")
    xt_im = nc.dram_tensor("xt_im", [D_IM, BSH], DT, kind="ExternalInput")
    xt_cd = nc.dram_tensor("xt_cd", [D_T, BSH], DT, kind="ExternalInput")
    wt = nc.dram_tensor("wt", [D_T, 320], DT, kind="ExternalInput")  # D pad 320
    wim = nc.dram_tensor("wim", [D_IM, 320], DT, kind="ExternalInput")
    # A^T per branch (rows = contraction dim of the w-gemm)
    amat_t = nc.dram_tensor("amat_t", [D, 320], DT, kind="ExternalInput")
    amat_i = nc.dram_tensor("amat_i", [D, 320], DT, kind="ExternalInput")
    amat_cd = nc.dram_tensor("amat_cd", [D, 320], DT, kind="ExternalInput")
    # packed per-out-dim columns: bt, bim, vT, vI, vCD
    cols = nc.dram_tensor("cols", [D, 5], F32, kind="ExternalInput")
    consts = nc.dram_tensor("consts", [1, 4], F32, kind="ExternalInput")
    onesd = nc.dram_tensor("onesd", [128, 1], DT, kind="ExternalInput")
    o_t = nc.dram_tensor("o_t", [D, BSH], DT, kind="ExternalOutput")
    o_im = nc.dram_tensor("o_im", [D, BSH], DT, kind="ExternalOutput")
    o_cd = nc.dram_tensor("o_cd", [D, BSH], DT, kind="ExternalOutput")

    ID = mybir.ActivationFunctionType.Identity
    SIG = mybir.ActivationFunctionType.Sigmoid
    ADD = mybir.AluOpType.add
    MUL = mybir.AluOpType.mult

    with tile.TileContext(nc) as tc, ExitStack() as ctx:
        singles = ctx.enter_context(tc.tile_pool(name="singles", bufs=1))
        sx = ctx.enter_context(tc.tile_pool(name="sx", bufs=1))
        sp = ctx.enter_context(tc.tile_pool(name="sp", bufs=1))
        ps = ctx.enter_context(tc.tile_pool(name="ps", bufs=1, space="PSUM"))

        # ---- persistent weights/constants ----
        wt_sb = singles.tile([128, KT, 320], DT)
        for k in range(KT):
            nc.sync.dma_start(out=wt_sb[:, k, :], in_=wt[k * 128:(k + 1) * 128, :])
        wim_sb = singles.tile([128, KI, 320], DT)
        for k in range(KI):
            nc.sync.dma_start(out=wim_sb[:, k, :], in_=wim[k * 128:(k + 1) * 128, :])
        a_sbs = {}
        for nm, dram in (("t", amat_t), ("i", amat_i), ("cd", amat_cd)):
            a_sb = singles.tile([128, 3, 320], DT, name=f"a_sb_{nm}")
            for j, (m0, m1) in enumerate(MCH):
                nc.sync.dma_start(out=a_sb[: m1 - m0, j, :], in_=dram[m0:m1, :])
            a_sbs[nm] = a_sb
        cols_sb = singles.tile([128, 3, 5], F32)
        for j, (m0, m1) in enumerate(MCH):
            nc.sync.dma_start(out=cols_sb[: m1 - m0, j, :], in_=cols[m0:m1, :])
        consts_sb = singles.tile([1, 4], F32)
        nc.sync.dma_start(out=consts_sb, in_=consts[:, :])
        ones_col = singles.tile([128, 1], DT)
        nc.sync.dma_start(out=ones_col, in_=onesd[:, 0:1])
        ones_row = singles.tile([1, 128], DT)
        nc.sync.dma_start(out=ones_row, in_=onesd[:, 0:1].rearrange("a b -> b a"))

        def load_x_pairs(dram, dim, t, tag, bufs):
            """Load [dim, NB] slice of column-tile t as (dim//256) tiles of
            [128, 2, NB] (two 128-row chunks per DMA)."""
            b0 = t * NB
            tiles = []
            for kp in range(dim // 256):
                xk = sx.tile([128, 2, NB], DT, tag=tag, bufs=bufs,
                             name=f"x_{tag}{kp}_{t}")
                src = dram[kp * 256:(kp + 1) * 256, b0:b0 + NB]
                nc.sync.dma_start(out=xk, in_=src.rearrange("(two p) n -> p two n", p=128))
                tiles.append(xk)
            return tiles

        def copy_out_p(pps_list, bias_ap_j, nm, t):
            p_sbs = []
            for j, (m0, m1) in enumerate(MCH):
                msz = m1 - m0
                p_sb = sp.tile([msz, NB], DT, tag=f"p_{nm}{j}", bufs=3,
                               name=f"p_{nm}{j}_{t}")
                nc.scalar.activation(out=p_sb, in_=pps_list[j], func=ID,
                                     bias=cols_sb[:msz, j, bias_ap_j:bias_ap_j + 1],
                                     scale=1.0)
                p_sbs.append(p_sb)
            return p_sbs

        def proj_im(x_tiles, t):
            """p_IM^T[300, NB] = Wim^T @ x^T + b, 3 m-chunks in SBUF.

            Chunk 2 (44 cols) is computed as two independent k-half sums,
            col-tiled onto disjoint PE column groups (half the slots), and
            recombined during copy-out."""
            pps = []
            for j in range(2):
                m0, m1 = MCH[j]
                pps.append(ps.tile([m1 - m0, NB], F32, tag="pps", bufs=6,
                                   name=f"pps_i{j}_{t}"))
            for j, (m0, m1) in enumerate(MCH[:2]):
                for k in range(KI):
                    rhs = x_tiles[k // 2][:, k % 2, :]
                    nc.tensor.matmul(pps[j], lhsT=wim_sb[:, k, m0:m1], rhs=rhs,
                                     start=(k == 0), stop=(k == KI - 1))
            if PAIR:
                pairI = ps.tile([128, NB], F32, tag="pps", bufs=6,
                                name=f"ppsI2_{t}")
                KH = KI // 2
                for kh in range(KH):
                    ka, kb = kh, kh + KH
                    ra = x_tiles[ka // 2][:, ka % 2, :]
                    rb = x_tiles[kb // 2][:, kb % 2, :]
                    st, sp_ = (kh == 0), (kh == KH - 1)
                    nc.tensor.matmul(pairI[0:64, :],
                                     lhsT=wim_sb[:, ka, 256:320], rhs=ra,
                                     start=st, stop=sp_, tile_position=(0, 0))
                    nc.tensor.matmul(pairI[64:128, :],
                                     lhsT=wim_sb[:, kb, 256:320], rhs=rb,
                                     start=st, stop=sp_, tile_position=(0, 64))
                # recombine halves + bias during copy-out
                p_sbs = []
                for j in range(2):
                    m0, m1 = MCH[j]
                    msz = m1 - m0
                    p_sb = sp.tile([msz, NB], DT, tag=f"p_i{j}", bufs=3,
                                   name=f"p_i{j}_{t}")
                    nc.scalar.activation(out=p_sb, in_=pps[j], func=ID,
                                         bias=cols_sb[:msz, j, 1:2], scale=1.0)
                    p_sbs.append(p_sb)
                tmph = sp.tile([44, NB], DT, tag="tmph", bufs=2,
                               name=f"tmph_{t}")
                nc.scalar.activation(out=tmph, in_=pairI[64:108, :], func=ID,
                                     bias=0.0, scale=1.0)
                p_sb2 = sp.tile([44, NB], DT, tag="p_i2", bufs=3,
                                name=f"p_i2_{t}")
                nc.vector.scalar_tensor_tensor(out=p_sb2, in0=pairI[0:44, :],
                                               scalar=cols_sb[:44, 2, 1:2],
                                               in1=tmph, op0=ADD, op1=ADD)
                p_sbs.append(p_sb2)
                return p_sbs
            pps.append(ps.tile([44, NB], F32, tag="pps", bufs=6,
                               name=f"pps_i2_{t}"))
            for k in range(KI):
                rhs = x_tiles[k // 2][:, k % 2, :]
                nc.tensor.matmul(pps[2], lhsT=wim_sb[:, k, 256:300], rhs=rhs,
                                 start=(k == 0), stop=(k == KI - 1))
            return copy_out_p(pps, 1, "i", t)

        def proj_tcd(x_t, x_cd, t):
            """Fused T & CD projections sharing Wt; third chunks col-tiled
            into one PSUM tile and issued back-to-back so they run
            concurrently on disjoint PE column groups."""
            if PAIR:
                pT = [ps.tile([128, NB], F32, tag="pps", bufs=6,
                              name=f"ppsT{j}_{t}") for j in range(2)]
                pC = [ps.tile([128, NB], F32, tag="pps", bufs=6,
                              name=f"ppsC{j}_{t}") for j in range(2)]
                pair = ps.tile([128, NB], F32, tag="pps", bufs=6,
                               name=f"ppsP_{t}")
                for j in range(2):
                    m0, m1 = MCH[j]
                    for k in range(KT):
                        rt = x_t[k // 2][:, k % 2, :]
                        st, sp_ = (k == 0), (k == KT - 1)
                        nc.tensor.matmul(pT[j], lhsT=wt_sb[:, k, m0:m1],
                                         rhs=rt, start=st, stop=sp_)
                for j in range(2):
                    m0, m1 = MCH[j]
                    for k in range(KT):
                        rc = x_cd[k // 2][:, k % 2, :]
                        st, sp_ = (k == 0), (k == KT - 1)
                        nc.tensor.matmul(pC[j], lhsT=wt_sb[:, k, m0:m1],
                                         rhs=rc, start=st, stop=sp_)
                # pair block: col-tiled 64-wide chunk2 MMs kept contiguous to
                # avoid per-MM mode transitions (~115ns each)
                for k in range(KT):
                    rt = x_t[k // 2][:, k % 2, :]
                    rc = x_cd[k // 2][:, k % 2, :]
                    st, sp_ = (k == 0), (k == KT - 1)
                    nc.tensor.matmul(pair[0:64, :], lhsT=wt_sb[:, k, 256:320],
                                     rhs=rt, start=st, stop=sp_,
                                     tile_position=(0, 0))
                    nc.tensor.matmul(pair[64:128, :], lhsT=wt_sb[:, k, 256:320],
                                     rhs=rc, start=st, stop=sp_,
                                     tile_position=(0, 64))
                p_t = copy_out_p([pT[0], pT[1], pair[0:44, :]], 0, "t", t)
                p_cd = copy_out_p([pC[0], pC[1], pair[64:108, :]], 0, "c", t)
            else:
                pT = [ps.tile([m1 - m0, NB], F32, tag="pps", bufs=6,
                              name=f"ppsT{j}_{t}")
                      for j, (m0, m1) in enumerate(MCH)]
                pC = [ps.tile([m1 - m0, NB], F32, tag="pps", bufs=6,
                              name=f"ppsC{j}_{t}")
                      for j, (m0, m1) in enumerate(MCH)]
                for pX, xs in ((pT, x_t), (pC, x_cd)):
                    for j, (m0, m1) in enumerate(MCH):
                        for k in range(KT):
                            rr = xs[k // 2][:, k % 2, :]
                            st, sp_ = (k == 0), (k == KT - 1)
                            nc.tensor.matmul(pX[j], lhsT=wt_sb[:, k, m0:m1],
                                             rhs=rr, start=st, stop=sp_)
                p_t = copy_out_p(pT, 0, "t", t)
                p_cd = copy_out_p(pC, 0, "c", t)
            return p_t, p_cd

        def w_chunks01(p_sbs, a_sb, v_j, nm, t, c0, c1):
            """w-gemm chunks 0,1 (full 128-col) + the (w+v)*p products."""
            nbc = c1 - c0
            msbs = []
            for j in (0, 1):
                m0, m1 = MCH[j]
                msz = m1 - m0
                wps = ps.tile([msz, nbc], F32, tag="wps", bufs=2,
                              name=f"wps_{nm}{j}_{t}_{c0}")
                for kk, (k0, k1) in enumerate(MCH):
                    nc.tensor.matmul(wps, lhsT=a_sb[: k1 - k0, kk, m0:m1],
                                     rhs=p_sbs[kk][:, c0:c1],
                                     start=(kk == 0), stop=(kk == 2))
                m_sb = sp.tile([msz, nbc], DT, tag=f"m{j}", bufs=2,
                               name=f"m_{nm}{j}_{t}_{c0}")
                nc.vector.scalar_tensor_tensor(out=m_sb, in0=wps,
                                               scalar=cols_sb[:msz, j, v_j:v_j + 1],
                                               in1=p_sbs[j][:, c0:c1],
                                               op0=ADD, op1=MUL)
                msbs.append(m_sb)
            return msbs

        def finish_z(msbs, m2, c_j, nm, t, nbc, c0):
            sum_sb = sp.tile([128, nbc], DT, tag="sum", bufs=3,
                             name=f"sum_{nm}_{t}_{c0}")
            nc.vector.tensor_add(sum_sb, msbs[0], msbs[1])
            nc.vector.tensor_add(sum_sb[:44, :], sum_sb[:44, :], m2)
            return sum_sb

        def epilogue(p_t, p_cd, p_im, t, c0=0, c1=NB):
            b0 = t * NB + c0
            nbc = c1 - c0
            ms = {}
            ms["t"] = w_chunks01(p_t, a_sbs["t"], 2, "t", t, c0, c1)
            ms["i"] = w_chunks01(p_im, a_sbs["i"], 3, "i", t, c0, c1)
            ms["c"] = w_chunks01(p_cd, a_sbs["cd"], 4, "c", t, c0, c1)

            # w-gemm chunk2: T & I col-tiled into one PSUM pair (contiguous
            # block), CD as plain 44-col MMs.
            m0, m1 = MCH[2]
            msz = m1 - m0
            if PAIR:
                pw = ps.tile([128, nbc], F32, tag="wps", bufs=2,
                             name=f"pw_{t}_{c0}")
                for kk, (k0, k1) in enumerate(MCH):
                    nc.tensor.matmul(pw[0:64, :],
                                     lhsT=a_sbs["t"][: k1 - k0, kk, 256:320],
                                     rhs=p_t[kk][:, c0:c1],
                                     start=(kk == 0), stop=(kk == 2),
                                     tile_position=(0, 0))
                    nc.tensor.matmul(pw[64:128, :],
                                     lhsT=a_sbs["i"][: k1 - k0, kk, 256:320],
                                     rhs=p_im[kk][:, c0:c1],
                                     start=(kk == 0), stop=(kk == 2),
                                     tile_position=(0, 64))
                w_t2 = pw[0:44, :]
                w_i2 = pw[64:108, :]
            else:
                wt2 = ps.tile([msz, nbc], F32, tag="wps", bufs=2,
                              name=f"wt2_{t}_{c0}")
                wi2 = ps.tile([msz, nbc], F32, tag="wps", bufs=2,
                              name=f"wi2_{t}_{c0}")
                for kk, (k0, k1) in enumerate(MCH):
                    nc.tensor.matmul(wt2, lhsT=a_sbs["t"][: k1 - k0, kk, m0:m1],
                                     rhs=p_t[kk][:, c0:c1],
                                     start=(kk == 0), stop=(kk == 2))
                for kk, (k0, k1) in enumerate(MCH):
                    nc.tensor.matmul(wi2, lhsT=a_sbs["i"][: k1 - k0, kk, m0:m1],
                                     rhs=p_im[kk][:, c0:c1],
                                     start=(kk == 0), stop=(kk == 2))
                w_t2, w_i2 = wt2, wi2
            wc2 = ps.tile([msz, nbc], F32, tag="wps", bufs=2,
                          name=f"wc2_{t}_{c0}")
            for kk, (k0, k1) in enumerate(MCH):
                nc.tensor.matmul(wc2, lhsT=a_sbs["cd"][: k1 - k0, kk, m0:m1],
                                 rhs=p_cd[kk][:, c0:c1],
                                 start=(kk == 0), stop=(kk == 2))

            m2s = {}
            for nm, wsrc, psrc, v_j in (("t", w_t2, p_t[2], 2),
                                        ("i", w_i2, p_im[2], 3),
                                        ("c", wc2, p_cd[2], 4)):
                m_sb = sp.tile([msz, nbc], DT, tag="m2", bufs=3,
                               name=f"m_{nm}2_{t}_{c0}")
                nc.vector.scalar_tensor_tensor(out=m_sb, in0=wsrc,
                                               scalar=cols_sb[:msz, 2, v_j:v_j + 1],
                                               in1=psrc[:, c0:c1],
                                               op0=ADD, op1=MUL)
                m2s[nm] = m_sb

            sums = {nm: finish_z(ms[nm], m2s[nm], None, nm, t, nbc, c0)
                    for nm in ("t", "i", "c")}

            # alphas: three M=1 ones-matmuls packed on distinct col groups
            al = ps.tile([65, nbc], F32, tag="wps", bufs=2, name=f"al_{t}_{c0}")
            if PAIR:
                for ci, nm in ((0, "t"), (32, "i"), (64, "c")):
                    nc.tensor.matmul(al[ci:ci + 1, :], lhsT=ones_col,
                                     rhs=sums[nm], start=True, stop=True,
                                     tile_position=(0, ci))
            else:
                al2 = ps.tile([1, nbc], F32, tag="wps", bufs=2, name=f"al2_{t}_{c0}")
                al3 = ps.tile([1, nbc], F32, tag="wps", bufs=2, name=f"al3_{t}_{c0}")
                als = {"t": al[0:1, :], "i": al2, "c": al3}
                for nm in ("t", "i", "c"):
                    nc.tensor.matmul(als[nm], lhsT=ones_col, rhs=sums[nm],
                                     start=True, stop=True)
            zs = {}
            for nm, ci, c_j in (("t", 0, 0), ("i", 32, 1), ("c", 64, 2)):
                z = sp.tile([1, nbc], DT, tag="rows", bufs=8,
                            name=f"z_{nm}_{t}_{c0}")
                src_al = al[ci:ci + 1, :] if PAIR else als[nm]
                nc.scalar.activation(out=z, in_=src_al, func=SIG,
                                     bias=consts_sb[0:1, c_j:c_j + 1],
                                     scale=INV_SQRT_D)
                zs[nm] = z
            z_t, z_i, z_cd = zs["t"], zs["i"], zs["c"]

            # d = (ZI - ZCD) * ZT ; a1 = sig(d) ; a2 = sig(-d)
            dz = sp.tile([1, nbc], DT, tag="rows", bufs=8, name=f"dz_{t}_{c0}")
            nc.vector.tensor_sub(dz, z_i, z_cd)
            nc.vector.tensor_mul(dz, dz, z_t)
            a1 = sp.tile([1, nbc], DT, tag="rows", bufs=8, name=f"a1_{t}_{c0}")
            a2 = sp.tile([1, nbc], DT, tag="rows", bufs=8, name=f"a2_{t}_{c0}")
            nc.scalar.activation(out=a1, in_=dz, func=SIG, bias=0.0, scale=1.0)
            nc.scalar.activation(out=a2, in_=dz, func=SIG, bias=0.0, scale=-1.0)

            for nm, (av, p_sbs, od) in (("i", (a1, p_im, o_im)),
                                        ("c", (a2, p_cd, o_cd))):
                ab = ps.tile([128, nbc], F32, tag="wps", bufs=2,
                             name=f"ab_{nm}_{t}_{c0}")
                nc.tensor.matmul(ab, lhsT=ones_row, rhs=av, start=True, stop=True)
                for j, (m0, m1) in enumerate(MCH):
                    msz = m1 - m0
                    o_sb = sp.tile([msz, nbc], DT, tag=f"o_{nm}{j}", bufs=3,
                                   name=f"o_{nm}{j}_{t}_{c0}")
                    nc.vector.tensor_mul(o_sb, ab[:msz, :], p_sbs[j][:, c0:c1])
                    nc.gpsimd.dma_start(out=od[m0:m1, b0:b0 + nbc], in_=o_sb)
            for j, (m0, m1) in enumerate(MCH):
                nc.gpsimd.dma_start(out=o_t[m0:m1, b0:b0 + nbc],
                                    in_=p_t[j][:, c0:c1])

        # Software pipeline: emit tile t's projection matmuls before tile
        # t-1's epilogue so the PE always has dense independent work queued.
        prev = None
        for t in range(NT):
            x_t = load_x_pairs(xt_t, D_T, t, "xt", 6)
            x_cd = load_x_pairs(xt_cd, D_T, t, "xc", 6)
            x_im = load_x_pairs(xt_im, D_IM, t, "xi", 12)
            p_t, p_cd = proj_tcd(x_t, x_cd, t)
            p_im = proj_im(x_im, t)
            if prev is not None:
                epilogue(*prev)
            prev = (p_t, p_cd, p_im, t)
        # last epilogue: two independent half-column chains pipeline against
        # each other, halving the exposed serial latency at the kernel tail
        epilogue(*prev, 0, NB // 2)
        epilogue(*prev, NB // 2, NB)

    nc.compile()
    return nc


def _get_nc():
    if "nc" not in _compiled:
        _compiled["nc"] = _build()
    return _compiled["nc"]


def kernel(T_feature, IM_feature, CD_feature, Wt, bt, Wim, bim,
           WqT, bqT, WkT, bkT, WqI, bqI, WkI, bkI, WqCD, bqCD, WkCD, bkCD):
    nc = _get_nc()

    f = np.asarray
    Wt = f(Wt, np.float32); bt = f(bt, np.float32)
    Wim = f(Wim, np.float32); bim = f(bim, np.float32)

    def fold(Wq, bq, Wk, bk):
        Wq = f(Wq, np.float64); bq = f(bq, np.float64)
        Wk = f(Wk, np.float64); bk = f(bk, np.float64)
        amat = np.zeros((D, 320), NPDT)
        amat[:, :D] = (Wk @ Wq.T).astype(NPDT)         # A^T, col-padded
        v = (Wk @ bq + Wq @ bk).astype(np.float32)
        c = float(bq @ bk)
        return amat, v, c

    amat_t, v_t, c_t = fold(WqT, bqT, WkT, bkT)
    amat_i, v_i, c_i = fold(WqI, bqI, WkI, bkI)
    amat_cd, v_cd, c_cd = fold(WqCD, bqCD, WkCD, bkCD)

    cols = np.stack([bt, bim, v_t, v_i, v_cd], axis=1).astype(np.float32)
    consts = np.array([[c_t * INV_SQRT_D, c_i * INV_SQRT_D,
                        c_cd * INV_SQRT_D, 0.0]], np.float32)
    ones = np.ones((128, 1), NPDT)

    xT = f(T_feature, np.float32).reshape(B, D_T)
    xI = f(IM_feature, np.float32).reshape(B, D_IM)
    xC = f(CD_feature, np.float32).reshape(B, D_T)

    Wt320 = np.zeros((D_T, 320), NPDT)
    Wt320[:, :D] = Wt.astype(NPDT)
    Wim320 = np.zeros((D_IM, 320), NPDT)
    Wim320[:, :D] = Wim.astype(NPDT)
    shared = {"wt": Wt320, "wim": Wim320, "amat_t": amat_t,
              "amat_i": amat_i, "amat_cd": amat_cd, "cols": cols,
              "consts": consts, "onesd": ones}
    in_maps = []
    for c in range(N_CORES):
        s = slice(c * BSH, (c + 1) * BSH)
        in_maps.append(dict(shared,
                            xt_t=xT[s].T.astype(NPDT),
                            xt_im=xI[s].T.astype(NPDT),
                            xt_cd=xC[s].T.astype(NPDT)))

    res = run_bass_kernel_spmd(nc, in_maps, core_ids=list(range(N_CORES)),
                               trace=bool(os.environ.get("KERNEL_TRACE")))
    if os.environ.get("KERNEL_TRACE"):
        print(f"HW exec time: {res.exec_time_ns} ns")

    outs = []
    for name in ("o_t", "o_im", "o_cd"):
        full = np.concatenate(
            [res.results[c][name].astype(np.float32) for c in range(N_CORES)],
            axis=1)                                        # [300, B]
        outs.append(np.ascontiguousarray(full.T)[:, None, :])  # [B, 1, 300]
    return tuple(outs)
